# revision 12
# baseline (speedup 1.0000x reference)
"""Trainium2 Bass kernel for the TSM-style gated segment-attention block.

Computation (per full batch of nt=128 frames = 16 clips x 8 segments):
  q = mean_hw(relu(bn(conv1x1_q(x))))      (nt, 32)
  k = mean_hw(relu(bn(conv1x1_k(x))))      (nt, 32)
  att = softmax_axis1(-q @ q^T per clip)   (16, 8, 8)
  qu  = att @ k + k                        (nt, 32)
  gate = sigmoid(relu(bn(qu @ wi^T + bi))) (nt, 256)
  out = gate[:, :, None, None] * x         (nt, 256, 28, 28)

Sharding: data-parallel over clips; 16 frames (2 whole clips) per core on
8 cores, params replicated.  Attention is clip-local so no collectives.

Key device-side tricks:
  - conv bias + BN (eval) + 1/784 mean divisor folded into one per-channel
    scale/bias applied by a single ACT op (relu) whose accum_out produces
    the spatial sum, i.e. the pooled q/k values, for free.
  - channel-PAIR layout: partition p holds channels 2p and 2p+1, which are
    contiguous in DRAM, so every frame's DMA is one 6272-byte descriptor
    per partition (half the queue items of the per-128 split layout).
  - q and k conv weights are concatenated into one [128, 64] stationary
    tile per channel-of-pair, so one pass over x computes both branches.
  - att = -q q^T is symmetric, so softmax over axis 1 (partition dim) is
    the transpose of the row softmax: compute the free-dim softmax R and
    use q_upd^T = v_frames^T @ R via one matmul with R as moving tensor.
  - sigmoid is computed as 1/(1+exp(-y)) (exact for y>=0) so the Scalar
    engine only ever loads the Exp activation table once; the
    Exp<->Sigmoid ACT_TABLE_LOAD pairs (1.5us each) otherwise land on the
    critical path of the second clip and starve the outbound DMA stream.
  - the final projection's bias bi is folded into the BN shift; gating is
    a per-partition tensor_scalar multiply into out tiles that are DMA'd
    straight out.
"""

from contextlib import ExitStack

import numpy as np

import concourse.bacc as bacc
import concourse.bass as bass
import concourse.mybir as mybir
import concourse.tile as tile
from concourse.bass_utils import run_bass_kernel_spmd

F32 = mybir.dt.float32
AF = mybir.ActivationFunctionType

N_CORES = 8
NT, C, H, W = 128, 256, 28, 28
HW = H * W                    # 784
NF = NT // N_CORES            # 16 frames per core
T = 8                         # segment (clip) length
NCLIP = NF // T               # 2 clips per core
C8 = 32                       # bottleneck channels
HALF = HW // 2                # 392, conv matmul N per psum chunk
CPK_COLS = 454                # packed-parameter tensor width
EPS = 1e-5

_CACHE: dict = {}


def _build_nc() -> bacc.Bacc:
    nc = bacc.Bacc()

    F32R = mybir.dt.float32r
    # x and the packed params are declared float32r (bit-identical to f32)
    # so the BIR verifier accepts them as single-pass-fp32 matmul inputs;
    # exact-math consumers use f32 bitcast views of the same bytes.
    x = nc.declare_dram_parameter("x", [NF, C, H, W], F32R, isOutput=False)
    # all small params packed into one tensor -> one DMA -> one semaphore
    # (per-instruction sync-wait slots are scarce: ACT allows only 2)
    cpk = nc.declare_dram_parameter("cpk", [128, CPK_COLS], F32R, isOutput=False)
    out = nc.declare_dram_parameter("out", [NF, C, H, W], F32, isOutput=True)

    # DRAM views: frame n as [128 partitions, (t, hw)] where partition p,
    # sub-chunk t holds channel 2p+t.  The pair (2p, 2p+1) is contiguous in
    # DRAM, so each partition's 1568 floats are a single 6272B run (one
    # descriptor).  One frame per trigger: larger trigger batches and
    # splitting across the Act HWDGE ring both destabilize the shared
    # physical DMA queues (measured 88-100us vs 76us).
    xv = x.rearrange("n (p t) h w -> n p (t h w)", p=128)
    ov = out.rearrange("n (p t) h w -> n p (t h w)", p=128)

    with tile.TileContext(nc) as tc:
        with ExitStack() as ctx:
            const = ctx.enter_context(tc.tile_pool(name="const", bufs=1))
            xpool = ctx.enter_context(tc.tile_pool(name="x", bufs=NF))
            scr = ctx.enter_context(tc.tile_pool(name="scr", bufs=3))
            small = ctx.enter_context(tc.tile_pool(name="small", bufs=2))
            gates = ctx.enter_context(tc.tile_pool(name="gates", bufs=2 * NCLIP))
            outp = ctx.enter_context(tc.tile_pool(name="outp", bufs=10))
            cps = ctx.enter_context(tc.tile_pool(name="cps", bufs=3, space="PSUM"))
            sps = ctx.enter_context(tc.tile_pool(name="sps", bufs=2, space="PSUM"))

            # ---- replicated parameters (single packed DMA) ----
            cpkt = const.tile([128, CPK_COLS], F32R)
            nc.sync.dma_start(cpkt[:], cpk[:])
            cpf = cpkt[:].bitcast(F32)       # exact-f32 view of same bytes
            w0r = cpkt[:, 0:64]              # row p = channel 2p   (q|k)
            w1r = cpkt[:, 64:128]            # row p = channel 2p+1 (q|k)
            identt = cpf[0:2 * C8, 128:192]
            wiTt = cpf[0:C8, 192:448]        # halves t: col p = chan 2p+t
            sqkt = cpf[0:2 * C8, 448:449]
            tqkt = cpf[0:2 * C8, 449:450]
            sit = cpf[:, 450:452]            # sit[p, t] = s_i[2p+t]
            tit = cpf[:, 452:454]

            # ---- phase 1: stream x in, conv+bn+relu+pool every frame ----
            pooleds = []
            for b in range(NCLIP):
                # pooled[c, f]: q rows 0:32, k(v) rows 32:64; written one
                # column per frame by the ACT accum_out
                pooleds.append(small.tile([2 * C8, T], F32,
                                          name=f"pooled{b}", tag=f"pooled{b}"))
            xts: list = [None] * NF
            for n in range(NF):
                b, fl = divmod(n, T)
                xt = xpool.tile([128, 2, HW], F32R, tag="x")
                xts[n] = xt
                nc.sync.dma_start(xt[:], xv[n])
                xr = xt            # native float32r view for the PE

                # [64, 1024] spans 2 PSUM banks; chunk A in bank 0 cols
                # 0:392, chunk B in bank 1 cols 512:904
                ps = cps.tile([2 * C8, 1024], F32, tag="cps")
                nc.tensor.matmul(ps[:, 0:HALF], w0r, xr[:, 0, 0:HALF],
                                 start=True, stop=False)
                nc.tensor.matmul(ps[:, 512:512 + HALF], w0r, xr[:, 0, HALF:HW],
                                 start=True, stop=False)
                nc.tensor.matmul(ps[:, 0:HALF], w1r, xr[:, 1, 0:HALF],
                                 start=False, stop=True)
                nc.tensor.matmul(ps[:, 512:512 + HALF], w1r, xr[:, 1, HALF:HW],
                                 start=False, stop=True)

                # relu(z*scale + bias) over both chunks in one op;
                # accum_out -> pooled mean (scale has the 1/784 divisor)
                psv = ps[:].rearrange("p (c h) -> p c h", c=2)[:, :, 0:HALF]
                sc0 = scr.tile([2 * C8, 2, HALF], F32, tag="scr")
                nc.scalar.activation(sc0[:], psv, AF.Relu,
                                     bias=tqkt, scale=sqkt,
                                     accum_out=pooleds[b][:, fl:fl + 1])

            # ---- phase 2: per-clip attention + gates ----
            gts_all = []
            for b in range(NCLIP):
                pooled = pooleds[b]
                # transpose -> [T, 64]; keep only the v half in SBUF
                trp = sps.tile([T, 2 * C8], F32, tag="sps")
                nc.tensor.transpose(trp[:], pooled[:], identt)
                vf = small.tile([T, C8], F32, tag="vf")
                nc.vector.tensor_copy(vf[:], trp[:, C8:2 * C8])

                # att_raw[i, j] = <q_i, q_j>  (symmetric)
                att = sps.tile([T, T], F32, tag="sps")
                nc.tensor.matmul(att[:], pooled[0:C8, :], pooled[0:C8, :],
                                 start=True, stop=True)

                # R = row-softmax(-att_raw): exp(-(z - rowmin)) / rowsum
                m8 = small.tile([T, 1], F32, tag="m8")
                nc.vector.tensor_reduce(m8[:], att[:],
                                        axis=mybir.AxisListType.X,
                                        op=mybir.AluOpType.min)
                e8 = small.tile([T, T], F32, tag="e8")
                s8 = small.tile([T, 1], F32, tag="s8")
                nc.scalar.activation(e8[:], att[:], AF.Exp,
                                     bias=m8[:], scale=-1.0, accum_out=s8[:])
                rinv = small.tile([T, 1], F32, tag="rinv")
                nc.vector.reciprocal(rinv[:], s8[:])
                rmat = small.tile([T, T], F32, tag="rmat")
                nc.vector.tensor_scalar_mul(rmat[:], e8[:], rinv[:])

                # q_upd^T[c, i] = sum_j v[j, c] * R[j, i]; then + v^T
                qups = sps.tile([C8, T], F32, tag="sps")
                nc.tensor.matmul(qups[:], vf[:], rmat[:], start=True, stop=True)
                qupd = small.tile([C8, T], F32, tag="qupd")
                nc.vector.tensor_add(qupd[:], qups[:], pooled[C8:2 * C8, :])

                # y^T[p, f] for channel 2p+t in half t; gate = sigmoid(
                # relu(bn)) computed as 1/(1+exp(-relu)) so no Sigmoid
                # ACT-table swap ever happens (Exp stays resident)
                gts = []
                for h in range(2):
                    yps = sps.tile([128, T], F32, tag="sps")
                    nc.tensor.matmul(yps[:], wiTt[:, 128 * h:128 * (h + 1)],
                                     qupd[:], start=True, stop=True)
                    g1 = gates.tile([128, T], F32, tag="g1")
                    nc.scalar.activation(g1[:], yps[:], AF.Relu,
                                         bias=tit[:, h:h + 1],
                                         scale=sit[:, h:h + 1])
                    esg = small.tile([128, T], F32, tag="esg")
                    nc.scalar.activation(esg[:], g1[:], AF.Exp, scale=-1.0)
                    dsg = small.tile([128, T], F32, tag="dsg")
                    nc.vector.tensor_scalar_add(dsg[:], esg[:], 1.0)
                    gt = gates.tile([128, T], F32, tag="gate")
                    nc.vector.reciprocal(gt[:], dsg[:])
                    gts.append(gt)
                gts_all.append(gts)

            # ---- phase 3: gate the resident x tiles into out tiles ----
            # (not in-place: the BIR verifier forbids non-f32r writers to
            # locations an fp32r matmul reads).  The multiply reads the
            # exact-f32 view, so the output is exact.
            for n in range(NF):
                b, fl = divmod(n, T)
                gts = gts_all[b]
                xf = xts[n][:].bitcast(F32)   # exact-f32 read view
                ot = outp.tile([128, 2, HW], F32, tag="ot")
                nc.vector.tensor_scalar_mul(ot[:, 0, :], xf[:, 0, :],
                                            gts[0][:, fl:fl + 1])
                nc.vector.tensor_scalar_mul(ot[:, 1, :], xf[:, 1, :],
                                            gts[1][:, fl:fl + 1])
                nc.sync.dma_start(ov[n], ot[:])
    nc.finalize()  # Bacc: run reg-alloc + wait-splitting passes
    return nc


def _derived_params(inp: dict) -> dict:
    f32 = np.float32
    wq, bq, gq, betaq, mq, vq = (np.asarray(inp[k], f32) for k in
                                 ("wq", "bq", "gq", "betaq", "mq", "vq"))
    wk, bk, gk, betak, mk, vk = (np.asarray(inp[k], f32) for k in
                                 ("wk", "bk", "gk", "betak", "mk", "vk"))
    wi, bi, gi, betai, mi, vi = (np.asarray(inp[k], f32) for k in
                                 ("wi", "bi", "gi", "betai", "mi", "vi"))

    sq = gq / np.sqrt(vq + EPS)
    tq = (bq - mq) * sq + betaq
    sk = gk / np.sqrt(vk + EPS)
    tk = (bk - mk) * sk + betak
    inv = f32(1.0 / HW)
    sqk = (np.concatenate([sq, sk]) * inv).reshape(2 * C8, 1)
    tqk = (np.concatenate([tq, tk]) * inv).reshape(2 * C8, 1)

    s_i = gi / np.sqrt(vi + EPS)
    # device computes z = q_upd @ wi^T without bi:
    # bn(z + bi) = z*s_i + (bi - mi)*s_i + betai
    t_i = (bi - mi) * s_i + betai
    cpk = np.zeros((128, CPK_COLS), f32)
    # channel-pair layout: partition p <-> channels (2p, 2p+1)
    cpk[:, 0:64] = np.concatenate([wq[:, 0::2].T, wk[:, 0::2].T], axis=1)
    cpk[:, 64:128] = np.concatenate([wq[:, 1::2].T, wk[:, 1::2].T], axis=1)
    cpk[0:2 * C8, 128:192] = np.eye(2 * C8, dtype=f32)
    cpk[0:C8, 192:320] = wi[0::2, :].T
    cpk[0:C8, 320:448] = wi[1::2, :].T
    cpk[0:2 * C8, 448] = sqk[:, 0]
    cpk[0:2 * C8, 449] = tqk[:, 0]
    cpk[:, 450:452] = s_i.reshape(128, 2)
    cpk[:, 452:454] = t_i.reshape(128, 2)
    return {"cpk": cpk}


def kernel(**inputs) -> np.ndarray:
    x = np.ascontiguousarray(np.asarray(inputs["x"], np.float32))
    assert x.shape == (NT, C, H, W), x.shape

    if "nc" not in _CACHE:
        _CACHE["nc"] = _build_nc()
    nc = _CACHE["nc"]

    params = _derived_params(inputs)
    in_maps = [
        {"x": x[i * NF:(i + 1) * NF], **params} for i in range(N_CORES)
    ]

    def _run() -> np.ndarray:
        res = run_bass_kernel_spmd(nc, in_maps, list(range(N_CORES)))
        return np.concatenate([r["out"] for r in res.results], axis=0)

    # The kernel is deterministic, so two good executions are bitwise
    # identical.  Execute twice and compare to guard against the rare
    # sporadic bad execution observed on the shared device (~1 in 20);
    # on mismatch, take the majority of three.
    out1 = _run()
    out2 = _run()
    if np.array_equal(out1, out2):
        return out1
    out3 = _run()
    if np.array_equal(out1, out3) or np.array_equal(out2, out3):
        return out3
    return out1
